# revision 41
# baseline (speedup 1.0000x reference)
"""Trainium2 Bass kernel for nn_AttentionBlock (B=2, T=2048, D=1024, H=16,
Dh=64, Ff=4096), SPMD across 8 NeuronCores in one NEFF launch.

Sharding:
  - Phase 1+2 (QKV projection + attention): 2 heads per core. The alibi
    tensor (256 MiB total) is read bf16, 2 heads per core.
  - AllToAll (1 MiB/core, bf16) re-shards attention output from heads to
    tokens.
  - Phase 3 (out-proj + residual + LayerNorm + MLP): 512 tokens per core.

Numerics:
  - Matmuls on the q/k path use float32r (TF32-like, ~1.5e-4) fed straight
    from fp32 HBM data; bf16 elsewhere (weights, alibi).
  - Attention computes transposed scores S^T(k,q) in 1024-wide tiles:
    Q.K^T runs in float32r, ScalarE computes exp(PSUM)->bf16, and the
    host-precomputed exp(alibi) (bf16) multiplies in on VectorE
    (exp(s+a) = exp(s)*exp(a)); the softmax denominator falls out of a
    ones column appended to V in the attn@v matmul; 1/denom is broadcast
    across partitions with gpsimd.partition_broadcast. Alibi tiles are
    cached per (head, q-chunk) round so both batches share one load.
  - Host-side algebraic folds: 1/sqrt(Dh) into w_q, ln2_w into w_mlp_in,
    b_mlp_in via gelu's per-partition bias, b_mlp_out into a second copy
    of the residual.

kernel(**inputs) takes FULL unsharded inputs, returns the FULL output.
"""

import sys

for _p in ("/opt/trn_rl_repo", "/root/.axon_site/_ro/trn_rl_repo"):
    if _p not in sys.path:
        sys.path.insert(0, _p)

import numpy as np
import ml_dtypes

import concourse.bass as bass
import concourse.tile as tile
from concourse import bacc, mybir
from concourse.bass_utils import run_bass_kernel_spmd
from concourse.masks import make_identity

BF16 = ml_dtypes.bfloat16

B, T, D, H, Dh, FF = 2, 2048, 1024, 16, 64, 4096
NTOK = B * T            # 4096
NCORES = 8
CHUNK = NTOK // NCORES  # 512 tokens per core
HPC = H // NCORES       # 2 heads per core

F32 = mybir.dt.float32
F32R = mybir.dt.float32r
BF = mybir.dt.bfloat16
AF = mybir.ActivationFunctionType

_COMPILED = None


def _build(sim1=False):
    nc = bacc.Bacc("TRN2", target_bir_lowering=False, debug=False,
                   num_devices=1 if sim1 else NCORES)

    # ---- kernel I/O (per core) ----
    xT_io = nc.dram_tensor("xT", [D, NTOK], F32R, kind="ExternalInput").ap()
    wqkvT_io = nc.dram_tensor("wqkvT", [D, 384], F32R, kind="ExternalInput").ap()
    alibiT_io = nc.dram_tensor("alibiT", [HPC, T, T], BF, kind="ExternalInput").ap()
    w_outT_io = nc.dram_tensor("w_outT", [D, D], BF, kind="ExternalInput").ap()
    x_res_io = nc.dram_tensor("x_res", [CHUNK, D], F32, kind="ExternalInput").ap()
    x_res_b_io = nc.dram_tensor("x_res_b", [CHUNK, D], F32, kind="ExternalInput").ap()
    # packed as [p, ff, kk, fin] = w_mlp_in_eff[ff*128+fin, kk*128+p]
    w_inP_io = nc.dram_tensor("w_inP", [128, 32, 8, 128], BF, kind="ExternalInput").ap()
    b_inT_io = nc.dram_tensor("b_inT", [128, 32], F32, kind="ExternalInput").ap()
    w_mlp_outT_io = nc.dram_tensor("w_mlp_outT", [FF, D], BF, kind="ExternalInput").ap()
    out_io = nc.dram_tensor("out", [CHUNK, D], F32, kind="ExternalOutput").ap()

    # ---- internal DRAM ----
    cc_send = nc.dram_tensor("cc_send", [D, CHUNK], BF)
    cc_recv = nc.dram_tensor("cc_recv", [D, CHUNK], BF)

    KT = T // 128   # 16 k-tiles per batch

    with tile.TileContext(nc) as tc:
        with tc.tile_pool(name="consts", bufs=1) as consts:
            identb = consts.tile([128, 128], BF, tag="identb")
            make_identity(nc, identb[:])
            identf = consts.tile([128, 128], F32, tag="identf")
            make_identity(nc, identf[:])
            identr = consts.tile([128, 128], F32R, tag="identr")
            nc.vector.tensor_copy(identr[:], identf[:])


            with tc.tile_pool(name="qkv", bufs=1) as qkv:
                # per-batch q/k/v so batch-1 projection overlaps batch-0
                # attention without false dependencies
                qTs, kTs, vs = [], [], []
                for b in range(2):
                    qTb = qkv.tile([128, T], F32R, tag=f"qT{b}", name=f"qT{b}")
                    kTb = qkv.tile([128, T], F32R, tag=f"kT{b}", name=f"kT{b}")
                    vb = qkv.tile([128, 16, 2, 65], BF, tag=f"v{b}",
                                  name=f"v{b}")
                    nc.vector.memset(vb[:, :, :, 64:65], 1.0)
                    qTs.append(qTb); kTs.append(kTb); vs.append(vb)
                # yn[hl][b*2+qc] covers tokens [b*T + qc*1024, ...)
                yn = [[qkv.tile([64, 1024], BF, tag=f"yn{hl}_{i}",
                                name=f"yn{hl}_{i}") for i in range(4)]
                      for hl in range(2)]

                with tc.tile_pool(name="p1x", bufs=1) as p1x, \
                     tc.tile_pool(name="p1w", bufs=1) as p1w, \
                     tc.tile_pool(name="p1ps", bufs=4, space="PSUM") as p1ps, \
                     tc.tile_pool(name="p1t", bufs=3) as p1t, \
                     tc.tile_pool(name="p1pt", bufs=2, space="PSUM") as p1pt:
                    wq = []
                    for kk in range(8):
                        w = p1w.tile([128, 384], F32R, tag=f"wq{kk}")
                        nc.sync.dma_start(w[:], wqkvT_io[kk * 128:(kk + 1) * 128, :])
                        wq.append(w)
                    def proj_pass(b):
                        qT, kT, v_all = qTs[b], kTs[b], vs[b]
                        with nc.named_scope(f"qkvproj{b}"):
                            xts = [p1x.tile([128, 2048], F32R,
                                            tag=f"xt{kk}", name=f"xt{kk}_{b}")
                                   for kk in range(8)]
                            for cc4 in range(4):
                                for kk in range(8):
                                    nc.sync.dma_start(
                                        xts[kk][:, cc4 * 512:(cc4 + 1) * 512],
                                        xT_io[kk * 128:(kk + 1) * 128,
                                              b * 2048 + cc4 * 512:
                                              b * 2048 + (cc4 + 1) * 512])
                            for t in range(4):
                                for m in range(3):   # q, k, v
                                    ps = p1ps.tile([128, 512], F32, tag="proj",
                                                   name=f"proj{b}_{t}_{m}")
                                    for kk in range(8):
                                        nc.tensor.matmul(
                                            ps[:],
                                            wq[kk][:, m * 128:(m + 1) * 128],
                                            xts[kk][:, t * 512:(t + 1) * 512],
                                            start=(kk == 0), stop=(kk == 7))
                                    if m == 0:
                                        nc.vector.tensor_copy(
                                            qT[:, t * 512:(t + 1) * 512], ps[:])
                                    elif m == 1:
                                        nc.vector.tensor_copy(
                                            kT[:, t * 512:(t + 1) * 512], ps[:])
                                    else:
                                        vt = p1t.tile([128, 512], F32R, tag="vt",
                                                      name=f"vt{b}_{t}")
                                        nc.vector.tensor_copy(vt[:], ps[:])
                                        for j in range(4):
                                            ti = t * 4 + j
                                            pt = p1pt.tile([128, 128], F32R,
                                                           tag="pt",
                                                           name=f"pt{b}_{ti}")
                                            nc.tensor.transpose(
                                                pt[:],
                                                vt[:, j * 128:(j + 1) * 128],
                                                identr[:])
                                            nc.vector.tensor_copy(
                                                v_all[:, ti, :, 0:64],
                                                pt[:].rearrange(
                                                    "p (a b) -> p a b", a=2))

                    proj_pass(0)
                    proj_pass(1)

                with nc.named_scope("attn"), \
                     tc.tile_pool(name="alb", bufs=20) as albp, \
                     tc.tile_pool(name="exps", bufs=6) as expp, \
                     tc.tile_pool(name="sps", bufs=2, space="PSUM") as spsp, \
                     tc.tile_pool(name="yups", bufs=2, space="PSUM") as yupp, \
                     tc.tile_pool(name="nrm", bufs=3) as nrmp:
                    al_cache = {}

                    def attn_pass(hl, qc, b):
                        if (hl, qc) not in al_cache:
                            al_cache[(hl, qc)] = [
                                albp.tile([128, 1024], BF, tag="al",
                                          name=f"al{hl}_{qc}_{kt}")
                                for kt in range(KT)]
                        als = al_cache[(hl, qc)]
                        yu = yupp.tile([65, 1024], F32, tag="yu",
                                       name=f"yu{hl}_{qc}_{b}")
                        for kt in range(KT):
                            if b == 0:
                                nc.sync.dma_start(
                                    als[kt][:],
                                    alibiT_io[hl, kt * 128:(kt + 1) * 128,
                                              qc * 1024:(qc + 1) * 1024])
                            sp = spsp.tile([128, 1024], F32, tag="sp",
                                           name=f"sp{hl}_{qc}_{b}_{kt}")
                            for h2 in range(2):
                                nc.tensor.matmul(
                                    sp[:, h2 * 512:(h2 + 1) * 512],
                                    kTs[b][hl * 64:(hl + 1) * 64,
                                           kt * 128:(kt + 1) * 128],
                                    qTs[b][hl * 64:(hl + 1) * 64,
                                           qc * 1024 + h2 * 512:
                                           qc * 1024 + (h2 + 1) * 512],
                                    start=True, stop=True)
                            ex0 = expp.tile([128, 1024], BF, tag="ex0",
                                            name=f"ex0_{hl}_{qc}_{b}_{kt}")
                            nc.scalar.activation(ex0[:], sp[:], AF.Exp)
                            ex = expp.tile([128, 1024], BF, tag="ex",
                                           name=f"ex_{hl}_{qc}_{b}_{kt}")
                            nc.vector.tensor_mul(ex[:], ex0[:], als[kt][:])
                            for h2 in range(2):
                                nc.tensor.matmul(
                                    yu[:, h2 * 512:(h2 + 1) * 512],
                                    vs[b][:, kt, hl, :],
                                    ex[:, h2 * 512:(h2 + 1) * 512],
                                    start=(kt == 0), stop=(kt == KT - 1))
                        rec = nrmp.tile([1, 1024], F32, tag="rec",
                                        name=f"rec{hl}_{qc}_{b}")
                        nc.vector.reciprocal(rec[:], yu[64:65, :])
                        bc = nrmp.tile([64, 1024], F32, tag="bc",
                                       name=f"bc{hl}_{qc}_{b}")
                        nc.gpsimd.partition_broadcast(bc[:], rec[:])
                        nc.vector.tensor_mul(
                            yn[hl][b * 2 + qc][:], yu[0:64, :], bc[:])
                        i = b * 2 + qc
                        nc.sync.dma_start(
                            bass.AP(tensor=cc_send,
                                    offset=(2 * i * 128 + hl * 64) * 512,
                                    ap=[[512, 64], [128 * 512, 2], [1, 512]]),
                            yn[hl][i][:].rearrange("p (h c) -> p h c", h=2))

                    for hl in range(2):
                        for qc in range(2):
                            for b in range(2):
                                attn_pass(hl, qc, b)

                with nc.named_scope("a2a"):
                    if sim1:
                        nc.sync.dma_start(cc_recv[:], cc_send[:])
                    else:
                        nc.gpsimd.collective_compute(
                            "AllToAll", mybir.AluOpType.bypass,
                            replica_groups=[list(range(NCORES))],
                            ins=[cc_send[:]], outs=[cc_recv[:]])

            # ---------------- phase 3: out-proj + LN + MLP ----------------
            with nc.named_scope("mlp"), \
                 tc.tile_pool(name="p3w", bufs=1) as p3w, \
                 tc.tile_pool(name="p3acc", bufs=2, space="PSUM") as p3acc, \
                 tc.tile_pool(name="p3mo", bufs=4, space="PSUM") as p3mo, \
                 tc.tile_pool(name="p3pt", bufs=2, space="PSUM") as p3pt, \
                 tc.tile_pool(name="p3sb", bufs=1) as p3sb, \
                 tc.tile_pool(name="p3r", bufs=3) as p3r, \
                 tc.tile_pool(name="p3s", bufs=4) as p3s, \
                 tc.tile_pool(name="mlpw", bufs=8) as mlpw:
                wout = []
                yrT = p3w.tile([128, 8, 512], BF, tag="yrT")
                nc.scalar.dma_start(
                    yrT[:], bass.AP(tensor=cc_recv, offset=0,
                                    ap=[[512, 128], [128 * 512, 8], [1, 512]]))
                yrecv = [yrT[:, kk, :] for kk in range(8)]
                for kk in range(8):
                    wo = p3w.tile([128, D], BF, tag=f"wo{kk}")
                    nc.sync.dma_start(wo[:], w_outT_io[kk * 128:(kk + 1) * 128, :])
                    wout.append(wo)
                b_in = p3sb.tile([128, 32], F32, tag="b_in")
                nc.sync.dma_start(b_in[:], b_inT_io[:])

                y_sb = p3sb.tile([128, 4, D], F32, tag="y_sb")
                y2_sb = p3sb.tile([128, 4, D], F32, tag="y2_sb")
                x_res_r = x_res_io.rearrange("(t p) d -> p t d", p=128)
                x_res_b_r = x_res_b_io.rearrange("(t p) d -> p t d", p=128)
                for tt in range(4):
                    xr = p3r.tile([128, D], F32, tag="xr")
                    nc.sync.dma_start(xr[:], x_res_r[:, tt, :])
                    xrb = p3r.tile([128, D], F32, tag="xrb")
                    nc.sync.dma_start(xrb[:], x_res_b_r[:, tt, :])
                    for dc in range(2):
                        ps = p3acc.tile([128, 512], F32, tag="acc")
                        for kk in range(8):
                            nc.tensor.matmul(
                                ps[:], yrecv[kk][:, tt * 128:(tt + 1) * 128],
                                wout[kk][:, dc * 512:(dc + 1) * 512],
                                start=(kk == 0), stop=(kk == 7))
                        nc.vector.tensor_add(
                            y_sb[:, tt, dc * 512:(dc + 1) * 512], ps[:],
                            xr[:, dc * 512:(dc + 1) * 512])
                        nc.vector.tensor_add(
                            y2_sb[:, tt, dc * 512:(dc + 1) * 512], ps[:],
                            xrb[:, dc * 512:(dc + 1) * 512])

                # LayerNorm -> h_norm (bf16) -> transpose -> hT (D-major)
                hT = p3sb.tile([128, 8, 512], BF, tag="hT")
                for tt in range(4):
                    stats = p3s.tile([128, 2, 6], F32, tag="stats")
                    for g in range(2):
                        nc.vector.bn_stats(
                            stats[:, g, :],
                            y_sb[:, tt, g * 512:(g + 1) * 512])
                    mv = p3s.tile([128, 2], F32, tag="mv")
                    nc.vector.bn_aggr(mv[:], stats[:])
                    eps = p3s.tile([128, 1], F32, tag="eps")
                    nc.vector.memset(eps[:], 1e-5)
                    sd = p3s.tile([128, 1], F32, tag="sd")
                    nc.scalar.activation(sd[:], mv[:, 1:2], AF.Sqrt,
                                         bias=eps[:], scale=1.0)
                    rstd = p3s.tile([128, 1], F32, tag="rstd")
                    nc.vector.reciprocal(rstd[:], sd[:])
                    nb = p3s.tile([128, 1], F32, tag="nb")
                    nc.vector.tensor_mul(nb[:], mv[:, 0:1], rstd[:])
                    nb2 = p3s.tile([128, 1], F32, tag="nb2")
                    nc.scalar.mul(nb2[:], nb[:], -1.0)
                    hn = p3r.tile([128, D], BF, tag="hn")
                    nc.scalar.activation(hn[:], y_sb[:, tt, :], AF.Identity,
                                         bias=nb2[:], scale=rstd[:])
                    for dc in range(8):
                        pt = p3pt.tile([128, 128], BF, tag="pt3")
                        nc.tensor.transpose(
                            pt[:], hn[:, dc * 128:(dc + 1) * 128], identb[:])
                        nc.vector.tensor_copy(
                            hT[:, dc, tt * 128:(tt + 1) * 128], pt[:])

                # MLP in + gelu -> hmT (Ff-major bf16)
                hmT = p3sb.tile([128, 32, 512], BF, tag="hmT")
                for ff in range(32):
                    wi = mlpw.tile([128, 8, 128], BF, tag="wi")
                    nc.sync.dma_start(wi[:], w_inP_io[:, ff, :, :])
                    ps = p3acc.tile([128, 512], F32, tag="acc")
                    for kk in range(8):
                        nc.tensor.matmul(ps[:], wi[:, kk, :], hT[:, kk, :],
                                         start=(kk == 0), stop=(kk == 7))
                    nc.scalar.activation(hmT[:, ff, :], ps[:], AF.Gelu,
                                         bias=b_in[:, ff:ff + 1], scale=1.0)

                # MLP out + final residual
                out_r = out_io.rearrange("(t p) d -> p t d", p=128)
                for dc in range(2):
                    pss = [p3mo.tile([128, 512], F32, tag="mo",
                                     name=f"mo{dc}_{i}") for i in range(4)]
                    for ff in range(32):
                        wo2 = mlpw.tile([128, 512], BF, tag="wo2")
                        nc.sync.dma_start(
                            wo2[:], w_mlp_outT_io[ff * 128:(ff + 1) * 128,
                                                  dc * 512:(dc + 1) * 512])
                        for tt in range(4):
                            nc.tensor.matmul(
                                pss[tt][:],
                                hmT[:, ff, tt * 128:(tt + 1) * 128], wo2[:],
                                start=(ff == 0), stop=(ff == 31))
                    for tt in range(4):
                        fin = p3s.tile([128, 512], F32, tag="fin")
                        nc.vector.tensor_add(
                            fin[:], pss[tt][:],
                            y2_sb[:, tt, dc * 512:(dc + 1) * 512])
                        nc.sync.dma_start(
                            out_r[:, tt, dc * 512:(dc + 1) * 512], fin[:])

    nc.compile()
    return nc


def _host_prep(x, alibi, ln1_w, w_qkv, w_out, ln2_w, w_mlp_in, b_mlp_in,
               w_mlp_out, b_mlp_out):
    f32 = np.float32
    x = np.asarray(x, f32)
    x_flat = np.ascontiguousarray(x.reshape(NTOK, D))
    xT = np.ascontiguousarray(x_flat.T)
    w_qkv = np.asarray(w_qkv, f32)
    w_out = np.asarray(w_out, f32)
    w_mlp_in = np.asarray(w_mlp_in, f32)
    w_mlp_out = np.asarray(w_mlp_out, f32)
    b_mlp_in = np.asarray(b_mlp_in, f32)
    b_mlp_out = np.asarray(b_mlp_out, f32)
    ln2_w = np.asarray(ln2_w, f32)
    alibi = np.asarray(alibi, f32)

    w_outT = np.ascontiguousarray(w_out.T).astype(BF16)
    w_in_eff = w_mlp_in * ln2_w[None, :]          # (FF, D)
    # packed [p, ff, kk, fin] = w_in_eff[ff*128+fin, kk*128+p]
    w_inP = np.ascontiguousarray(
        w_in_eff.reshape(32, 128, 8, 128).transpose(3, 0, 2, 1)).astype(BF16)
    w_mlp_outT = np.ascontiguousarray(w_mlp_out.T).astype(BF16)
    b_inT = np.ascontiguousarray(b_mlp_in.reshape(32, 128).T)

    in_maps = []
    for c in range(NCORES):
        h0 = HPC * c
        qrows = w_qkv[h0 * Dh:(h0 + HPC) * Dh] / np.sqrt(np.float32(Dh))
        krows = w_qkv[H * Dh + h0 * Dh:H * Dh + (h0 + HPC) * Dh]
        vrows = w_qkv[2 * H * Dh + h0 * Dh:2 * H * Dh + (h0 + HPC) * Dh]
        wqkvT = np.ascontiguousarray(np.concatenate([qrows, krows, vrows], 0).T)
        alibiT = np.exp(np.ascontiguousarray(
            np.transpose(alibi[0, h0:h0 + HPC], (0, 2, 1)))).astype(BF16)
        x_res = np.ascontiguousarray(x_flat[c * CHUNK:(c + 1) * CHUNK])
        x_res_b = x_res + b_mlp_out[None, :]
        in_maps.append({
            "xT": xT, "wqkvT": wqkvT, "alibiT": alibiT, "w_outT": w_outT,
            "x_res": x_res, "x_res_b": x_res_b, "w_inP": w_inP,
            "b_inT": b_inT, "w_mlp_outT": w_mlp_outT,
        })
    return in_maps


def _get_compiled():
    global _COMPILED
    if _COMPILED is None:
        _COMPILED = _build()
    return _COMPILED


def kernel(_trace=False, **inputs):
    nc = _get_compiled()
    in_maps = _host_prep(**inputs)
    res = None
    for attempt in range(3):
        try:
            res = run_bass_kernel_spmd(nc, in_maps,
                                       core_ids=list(range(NCORES)),
                                       trace=_trace)
            break
        except Exception:
            if attempt == 2:
                raise
    out = np.concatenate([res.results[c]["out"] for c in range(NCORES)], 0)
    out = out.reshape(B, T, D).astype(np.float32)
    if _trace:
        return out, res
    return out
